# revision 1
# baseline (speedup 1.0000x reference)
"""DialogueEIN fused kernel for 8 TRN2 NeuronCores (data-parallel over batch).

Self-contained: hardcodes shapes for the nn_DialogueEIN problem
  x[64,256,512], T=256, H=512, NH=8 heads, E=7 emotion slots, window 5.

Strategy (per core, 8 batches, processed as 4 batch-PAIRS):
  - All activations live in "transposed" space [H, T] so attention scores are
    computed directly as S_T[k, j] (keys on partitions, queries on free dim):
    qT/kT come straight out of the projection matmuls; softmax needs no
    transposes anywhere.  Two batches share each tile on the free dim
    ([128, 512]) so projection/LN/exp instruction counts halve.
  - Softmax without max-subtraction: scores are O(1) and the additive mask
    bias is -50 instead of -1e4 (identical through softmax: fully-masked rows
    reduce to the reference's plain softmax; partially-masked rows leave
    masked weights at ~e^-48 relative -- below fp32 noise).
  - The PV matmul's lhsT is a contiguous [V_h | ones64] 128-column group, so
    PSUM rows 64:128 hold 64 broadcast copies of the softmax denominator row;
    normalization is a 64-lane reciprocal + the eviction multiply.
  - Mask biases for global/intra/inter are rank<=3 outer products accumulated
    into the score PSUM by tiny extra matmuls (host ships factor vectors).
    The local sliding-window branch uses a multiplicative post-exp mask
    built per pair from a constant band matrix (4 rank-1 matmuls + 2 DVE ops).
  - Host folds: b_Wo[i] @ W1_i (kills the concat+W1 matmul), ln2 gamma/beta
    into W2, t_bv/b_bv into downstream biases, 1/sqrt(dh) into Wq.
  - LayerNorm over the partition axis: ones-column matmuls for mean/E[x^2],
    PE rank-1 broadcast of rstd / (-mu*rstd) rows, per-partition gamma/beta.
  - All big matmuls run as float32r (full-rate fp32 mode on the PE).
"""

import numpy as np

import concourse.bass as bass
import concourse.mybir as mybir
import concourse.tile as tile
from concourse import bacc
from concourse.bass_utils import run_bass_kernel_spmd

F32 = mybir.dt.float32
F32R = mybir.dt.float32r
ALU = mybir.AluOpType
ACTF = mybir.ActivationFunctionType

B, T, H, NH, E = 64, 256, 512, 8, 7
DH = H // NH
NCORES = 8
BL = B // NCORES          # 8 batches per core
PR = BL // 2              # 4 batch-pairs per core
WD = 2 * T                # 512: paired free width
VW = NH * 128             # 1024: V_aug width ([V_h | ones64] per head)
NEG = -50.0
KT = H // 128             # 4
MT = T // 128             # 2
EPS = 1e-12

_CACHE = {}


def _build(apply_g1b1, apply_c2):
    nc = bacc.Bacc("TRN2", target_bir_lowering=False, debug=False,
                   enable_asserts=False)

    def din(name, shape, dt=F32R):
        return nc.dram_tensor(name, list(shape), dt, kind="ExternalInput").ap()

    xT = din("xT", (PR, H, WD))
    smalls = din("smalls", (BL, 3, 9, T))
    bandd = din("bandd", (T, WD), mybir.dt.bfloat16)
    kTemo = din("kTemo", (H, E))
    vemoaug = din("vemoaug", (E, VW))
    tWq = din("tWq", (H, H))
    tWo = din("tWo", (H, H))
    bWq = din("bWq", (4, H, H))
    bWk = din("bWk", (4, H, H))
    bWv = din("bWv", (4, H, H))
    What = din("What", (4, H, H))
    W2p = din("W2p", (H, H))
    onesd = din("onesd", (T,))
    tbq = din("tbq", (H,), F32)
    tbo = din("tbo", (H,), F32)
    g1 = din("g1", (H,), F32)
    b1v = din("b1v", (H,), F32)
    bbq = din("bbq", (4, H), F32)
    bbk = din("bbk", (4, H), F32)
    bhat = din("bhat", (H,), F32)
    c2row = din("c2row", (H,), F32)
    out = nc.dram_tensor("out", [BL, T, H], F32, kind="ExternalOutput").ap()

    with tile.TileContext(nc) as tc:
        cst = tc.alloc_tile_pool(name="cst", bufs=1)
        per = tc.alloc_tile_pool(name="per", bufs=1)
        wts = tc.alloc_tile_pool(name="wts", bufs=1)
        act = tc.alloc_tile_pool(name="act", bufs=1)
        pmm = tc.alloc_tile_pool(name="pmm", bufs=3, space="PSUM")
        psc = tc.alloc_tile_pool(name="psc", bufs=3, space="PSUM")
        pcx = tc.alloc_tile_pool(name="pcx", bufs=2, space="PSUM")

        # ---- constants ----
        ones128 = cst.tile([128, 1], F32R, name="ones128")
        nc.sync.dma_start(out=ones128, in_=onesd[0:128])
        onr32 = cst.tile([1, 128], F32R, name="onr32")
        nc.sync.dma_start(out=onr32, in_=onesd[0:128])
        eps_t = cst.tile([1, 1], F32, name="eps_t")
        nc.vector.memset(eps_t, EPS)
        kTe = []
        for k in range(KT):
            t = cst.tile([128, E], F32R, name=f"kTemo{k}")
            nc.sync.dma_start(out=t, in_=kTemo[k * 128:(k + 1) * 128, :])
            kTe.append(t)
        vea = cst.tile([E, VW], F32R, name="vemoaug")
        nc.sync.dma_start(out=vea, in_=vemoaug)
        bandt = []
        for m in range(MT):
            t = cst.tile([128, WD], mybir.dt.bfloat16, name=f"band{m}")
            nc.sync.dma_start(out=t, in_=bandd[m * 128:(m + 1) * 128, :])
            bandt.append(t)

        def vec_tiles(v, nm):
            ts = []
            for k in range(KT):
                t = cst.tile([128, 1], F32, name=f"{nm}{k}")
                nc.sync.dma_start(out=t, in_=v[k * 128:(k + 1) * 128])
                ts.append(t)
            return ts

        tbq_t = vec_tiles(tbq, "tbq")
        tbo_t = vec_tiles(tbo, "tbo")
        bhat_t = vec_tiles(bhat, "bhat")
        g1_t = vec_tiles(g1, "g1") if apply_g1b1 else None
        b1_t = vec_tiles(b1v, "b1v") if apply_g1b1 else None
        bbq_t = [vec_tiles(bbq[i], f"bbq{i}") for i in range(4)]
        bbk_t = [vec_tiles(bbk[i], f"bbk{i}") for i in range(4)]
        c2n = None
        if apply_c2:
            c2n = cst.tile([128, H], F32, name="c2n")
            nc.sync.dma_start(
                out=c2n, in_=bass.AP(tensor=c2row.tensor, offset=c2row.offset,
                                     ap=[[0, 128], [1, H]]))

        # persistent per-pair state
        htT = [[per.tile([128, WD], F32R, name=f"htT_{p}_{k}")
                for k in range(KT)] for p in range(PR)]
        h2sb = [[per.tile([128, WD], F32R, name=f"h2sb_{p}_{k}")
                 for k in range(KT)] for p in range(PR)]

        def proj_T(wtiles, rhs_tiles, bias_tiles, tag):
            """[H, WD] = W.T @ rhs(pair), +bias per-partition (ACT evict)."""
            res = []
            for mo in range(KT):
                ps = pmm.tile([128, WD], F32, tag="mm", bufs=3, name="psp")
                for ki in range(KT):
                    nc.tensor.matmul(
                        ps, wtiles[ki][:, mo * 128:(mo + 1) * 128],
                        rhs_tiles[ki], start=(ki == 0), stop=(ki == KT - 1))
                s = act.tile([128, WD], F32R, tag=tag, bufs=4, name="proj")
                nc.scalar.activation(s, ps, ACTF.Identity, bias=bias_tiles[mo])
                res.append(s)
            return res

        def softmax_pv2(h, e_tiles, va2, ctxTn):
            """PV for BOTH batch halves of one head into one [128, WD] psum;
            lhsT = [V_h | ones64] contiguous group so psum rows 64:128 hold
            the denominator rows; one reciprocal + one multiply-evict."""
            ps = pcx.tile([128, WD], F32, tag="ctx", bufs=2, name="ctxps")
            nkt = len(e_tiles)
            for bh in range(2):
                hsl = slice(bh * T, (bh + 1) * T)
                for kt in range(nkt):
                    nc.tensor.matmul(ps[:, hsl],
                                     va2[bh][kt][:, h * 128:(h + 1) * 128],
                                     e_tiles[kt][:, hsl], start=(kt == 0),
                                     stop=(kt == nkt - 1))
            recD = act.tile([64, WD], F32, tag="recD", bufs=2, name="recD")
            nc.vector.reciprocal(out=recD, in_=ps[64:128, :])
            pi = (h % 2) * 64
            nc.vector.tensor_tensor(out=ctxTn[h // 2][pi:pi + 64, :],
                                    in0=ps[0:64, :], in1=recD, op=ALU.mult)

        def bcast_row(row_ap):
            t = pcx.tile([128, WD], F32, tag="ctx", bufs=2, name="bcast")
            nc.tensor.matmul(t, onr32, row_ap, start=True, stop=True)
            return t

        def layer_norm_T(s_tiles, gb, dests=None):
            """LN over the partition (H) axis of transposed [H, WD] pair."""
            psmu = psc.tile([128, WD], F32, tag="sc", bufs=3, name="psmu")
            for k in range(KT):
                nc.tensor.matmul(psmu[0:1, :], ones128, s_tiles[k],
                                 start=(k == 0), stop=(k == KT - 1))
            pss2 = psc.tile([128, WD], F32, tag="sc", bufs=3, name="pss2")
            for k in range(KT):
                sq = act.tile([128, WD], F32R, tag="sq", bufs=2, name="sq")
                nc.scalar.activation(sq, s_tiles[k], ACTF.Square)
                nc.tensor.matmul(pss2[0:1, :], ones128, sq,
                                 start=(k == 0), stop=(k == KT - 1))

            def stat():
                return act.tile([1, WD], F32R, tag="lnstat", bufs=5,
                                name="lnstat")
            mu, ex2, var, rstd, nm = (stat() for _ in range(5))
            nc.scalar.activation(mu, psmu[0:1, :], ACTF.Copy, scale=1.0 / H)
            nc.scalar.activation(ex2, pss2[0:1, :], ACTF.Copy, scale=1.0 / H)
            nc.vector.scalar_tensor_tensor(var, mu, -1.0, mu,
                                           op0=ALU.mult, op1=ALU.mult)
            nc.vector.tensor_tensor(out=var, in0=ex2, in1=var, op=ALU.add)
            nc.scalar.activation(var, var, ACTF.Sqrt, bias=eps_t)
            with nc.allow_low_precision("f32r rows feed broadcast matmuls"):
                nc.vector.reciprocal(rstd, var)
            nc.vector.scalar_tensor_tensor(nm, mu, -1.0, rstd,
                                           op0=ALU.mult, op1=ALU.mult)
            RS = bcast_row(rstd)
            NM = bcast_row(nm)
            res = []
            for k in range(KT):
                o = (dests[k] if dests is not None else
                     act.tile([128, WD], F32R, tag="lno", bufs=4, name="lno"))
                nc.vector.tensor_tensor(out=o, in0=s_tiles[k], in1=RS,
                                        op=ALU.mult)
                nc.vector.tensor_tensor(out=o, in0=o, in1=NM, op=ALU.add)
                if gb is not None:
                    nc.vector.tensor_scalar(o, o, gb[0][k], gb[1][k],
                                            op0=ALU.mult, op1=ALU.add)
                res.append(o)
            return res

        # ---------------- Stage T: tendency attention + LN1 ----------------
        wq_t = [wts.tile([128, H], F32R, tag="w", bufs=16, name="twq")
                for _ in range(KT)]
        wo_t = [wts.tile([128, H], F32R, tag="w", bufs=16, name="two")
                for _ in range(KT)]
        for k in range(KT):
            nc.sync.dma_start(out=wq_t[k], in_=tWq[k * 128:(k + 1) * 128, :])
            nc.sync.dma_start(out=wo_t[k], in_=tWo[k * 128:(k + 1) * 128, :])

        for p in range(PR):
            xTb = []
            for k in range(KT):
                t = act.tile([128, WD], F32R, tag="xT", bufs=4, name="xTt")
                nc.sync.dma_start(out=t, in_=xT[p, k * 128:(k + 1) * 128, :])
                xTb.append(t)
            qT = proj_T(wq_t, xTb, tbq_t, "qT")
            ctxTn = [act.tile([128, WD], F32R, tag="ctxTn", bufs=4,
                              name="ctxTn") for _ in range(KT)]
            for h in range(NH):
                pi = (h % 2) * 64
                sps = psc.tile([128, WD], F32, tag="sc", bufs=3, name="scte")
                nc.tensor.matmul(sps[0:E, :], kTe[h // 2][pi:pi + 64, :],
                                 qT[h // 2][pi:pi + 64, :],
                                 start=True, stop=True)
                et = act.tile([E, WD], F32R, tag="et", bufs=3, name="ett")
                nc.scalar.activation(et, sps[0:E, :], ACTF.Exp)
                softmax_pv2(h, [et[0:E, :]], [[vea], [vea]], ctxTn)
            s1 = []
            for mo in range(KT):
                ps = pmm.tile([128, WD], F32, tag="mm", bufs=3, name="psh")
                for ki in range(KT):
                    nc.tensor.matmul(ps, wo_t[ki][:, mo * 128:(mo + 1) * 128],
                                     ctxTn[ki], start=(ki == 0),
                                     stop=(ki == KT - 1))
                s = act.tile([128, WD], F32R, tag="s1", bufs=4, name="s1")
                nc.vector.scalar_tensor_tensor(s, ps, tbo_t[mo], xTb[mo],
                                               op0=ALU.add, op1=ALU.add)
                s1.append(s)
            layer_norm_T(s1, (g1_t, b1_t) if apply_g1b1 else None,
                         dests=htT[p])

        # ---------------- Branch stages ----------------
        for i in range(4):
            wq_b = [wts.tile([128, H], F32R, tag="w", bufs=16, name="bwq")
                    for _ in range(KT)]
            wk_b = [wts.tile([128, H], F32R, tag="w", bufs=16, name="bwk")
                    for _ in range(KT)]
            wv_b = [wts.tile([128, H], F32R, tag="w", bufs=16, name="bwv")
                    for _ in range(KT)]
            wh_b = [wts.tile([128, H], F32R, tag="w", bufs=16, name="bwh")
                    for _ in range(KT)]
            for k in range(KT):
                sl = slice(k * 128, (k + 1) * 128)
                nc.sync.dma_start(out=wq_b[k], in_=bWq[i, sl, :])
                nc.sync.dma_start(out=wk_b[k], in_=bWk[i, sl, :])
                nc.sync.dma_start(out=wv_b[k], in_=bWv[i, sl, :])
                nc.sync.dma_start(out=wh_b[k], in_=What[i, sl, :])
            gsl = {0: slice(0, 2), 1: slice(5, 9),
                   2: slice(1, 3), 3: slice(3, 5)}[i]
            ng = gsl.stop - gsl.start
            for p in range(PR):
                sms = []
                for bh in range(2):
                    sm = act.tile([3, 4, T], F32R, tag="sm", bufs=2, name="sm",
                                  padded_shape=None)
                    sm = sm[:, 0:ng, :]
                    nc.sync.dma_start(out=sm,
                                      in_=smalls[2 * p + bh][:, gsl, :])
                    sms.append(sm)
                ml = None
                if i == 1:  # local: multiplicative mask band*outer(col,row)+B
                    ml = []
                    for m in range(MT):
                        msl = slice(m * 128, (m + 1) * 128)
                        psA = psc.tile([128, WD], F32, tag="sc", bufs=3,
                                       name="psA")
                        psB = psc.tile([128, WD], F32, tag="sc", bufs=3,
                                       name="psB")
                        for bh in range(2):
                            hsl = slice(bh * T, (bh + 1) * T)
                            nc.tensor.matmul(psA[:, hsl], sms[bh][0:1, 0, msl],
                                             sms[bh][0:1, 1, :],
                                             start=True, stop=True)
                            nc.tensor.matmul(psB[:, hsl], sms[bh][0:1, 3, msl],
                                             sms[bh][0:1, 2, :],
                                             start=True, stop=True)
                        mt_ = act.tile([128, WD], F32, tag="ml", bufs=2,
                                       name="ml")
                        nc.vector.tensor_tensor(out=mt_, in0=psA,
                                                in1=bandt[m], op=ALU.mult)
                        nc.vector.tensor_tensor(out=mt_, in0=mt_, in1=psB,
                                                op=ALU.add)
                        ml.append(mt_)
                qT = proj_T(wq_b, htT[p], bbq_t[i], "qT")
                kT = proj_T(wk_b, htT[p], bbk_t[i], "kT")
                va = [[None] * MT for _ in range(2)]
                for bh in range(2):
                    for mo in range(MT):
                        ps = pmm.tile([128, H], F32, tag="mm", bufs=3,
                                      name="psv")
                        for ki in range(KT):
                            off = bh * T + mo * 128
                            nc.tensor.matmul(
                                ps, htT[p][ki][:, off:off + 128],
                                wv_b[ki], start=(ki == 0), stop=(ki == KT - 1))
                        vt = act.tile([128, VW], F32R, tag="va", bufs=4,
                                      name="va")
                        vt3 = vt.rearrange("p (h d) -> p h d", h=NH)
                        ps3 = ps.rearrange("p (h d) -> p h d", h=NH)
                        nc.scalar.activation(vt3[:, :, 0:64], ps3, ACTF.Copy)
                        nc.vector.memset(vt3[:, :, 64:128].bitcast(
                            mybir.dt.uint32), 0x3F800000)
                        va[bh][mo] = vt
                ctxTn = [act.tile([128, WD], F32R, tag="ctxTn", bufs=4,
                                  name="ctxTn") for _ in range(KT)]
                for h in range(NH):
                    pi = (h % 2) * 64
                    ets = []
                    for m in range(MT):
                        msl = slice(m * 128, (m + 1) * 128)
                        sps = psc.tile([128, WD], F32, tag="sc", bufs=3,
                                       name="scb")
                        for bh in range(2):
                            hsl = slice(bh * T, (bh + 1) * T)
                            nc.tensor.matmul(
                                sps[:, hsl],
                                kT[h // 2][pi:pi + 64, bh * T + m * 128:
                                           bh * T + m * 128 + 128],
                                qT[h // 2][pi:pi + 64, hsl],
                                start=True, stop=(i == 1))
                            if i == 0:
                                nc.tensor.matmul(sps[:, hsl],
                                                 sms[bh][0:1, 0, msl],
                                                 sms[bh][0:1, 1, :],
                                                 start=False, stop=True)
                            elif i in (2, 3):
                                nc.tensor.matmul(sps[:, hsl],
                                                 sms[bh][:, 1, msl],
                                                 sms[bh][:, 0, :],
                                                 start=False, stop=True)
                        et = act.tile([128, WD], F32R, tag="et", bufs=3,
                                      name="etb")
                        nc.scalar.activation(et, sps, ACTF.Exp)
                        if i == 1:
                            nc.vector.tensor_tensor(out=et, in0=et, in1=ml[m],
                                                    op=ALU.mult)
                        ets.append(et)
                    softmax_pv2(h, ets, va, ctxTn)
                for mo in range(KT):
                    ps = pmm.tile([128, WD], F32, tag="mm", bufs=3,
                                  name="psh2")
                    for ki in range(KT):
                        nc.tensor.matmul(
                            ps, wh_b[ki][:, mo * 128:(mo + 1) * 128],
                            ctxTn[ki], start=(ki == 0), stop=(ki == KT - 1))
                    if i == 0:
                        nc.scalar.copy(out=h2sb[p][mo], in_=ps)
                    elif i < 3:
                        nc.vector.tensor_tensor(out=h2sb[p][mo],
                                                in0=h2sb[p][mo], in1=ps,
                                                op=ALU.add)
                    else:
                        nc.vector.scalar_tensor_tensor(
                            h2sb[p][mo], ps, bhat_t[mo], h2sb[p][mo],
                            op0=ALU.add, op1=ALU.add)

        # ---------------- Final: residual + LN2 + W2 ----------------
        w2_t = [wts.tile([128, H], F32R, tag="w", bufs=16, name="w2p")
                for _ in range(KT)]
        for k in range(KT):
            nc.sync.dma_start(out=w2_t[k], in_=W2p[k * 128:(k + 1) * 128, :])
        for p in range(PR):
            for k in range(KT):
                nc.vector.tensor_tensor(out=h2sb[p][k], in0=h2sb[p][k],
                                        in1=htT[p][k], op=ALU.add)
            n2 = layer_norm_T(h2sb[p], None)
            for bh in range(2):
                for mo in range(MT):
                    ps = pmm.tile([128, H], F32, tag="mm", bufs=3, name="pso")
                    for ki in range(KT):
                        off = bh * T + mo * 128
                        nc.tensor.matmul(ps, n2[ki][:, off:off + 128],
                                         w2_t[ki], start=(ki == 0),
                                         stop=(ki == KT - 1))
                    osb = act.tile([128, H], F32, tag="osb", bufs=1,
                                   name="osb")
                    if apply_c2:
                        nc.vector.tensor_tensor(out=osb, in0=ps, in1=c2n,
                                                op=ALU.add)
                    else:
                        nc.scalar.copy(out=osb, in_=ps)
                    nc.sync.dma_start(
                        out=out[2 * p + bh, mo * 128:(mo + 1) * 128, :],
                        in_=osb)
        pcx.release()
        psc.release()
        pmm.release()
        act.release()
        wts.release()
        per.release()
        cst.release()

    nc.compile()
    return nc


def _host_prep(inputs):
    f32 = np.float32
    g = {}
    x = np.asarray(inputs["x"], f32)
    lengths = np.asarray(inputs["lengths"])
    speakers = np.asarray(inputs["speakers"])
    emo = np.asarray(inputs["emo_table"], f32)

    xTa = np.ascontiguousarray(x.transpose(0, 2, 1))  # [B, H, T]
    xTp = np.ascontiguousarray(
        xTa.reshape(B // 2, 2, H, T).transpose(0, 2, 1, 3).reshape(
            B // 2, H, WD))
    j = np.arange(T)
    row = (j[None, :] < lengths[:, None]).astype(f32)
    col = row
    sp = speakers.astype(f32)
    u1 = row * sp
    u2 = row * (1.0 - sp)
    ones = np.ones_like(row)
    z = np.zeros_like(row)
    sm = np.zeros((B, 3, 9, T), f32)
    sm[:, 0, 0] = NEG * (1.0 - col)                               # 0: FR glob
    sm[:, 0, 1], sm[:, 1, 1], sm[:, 2, 1] = ones, u1, u2          # 1: FL
    sm[:, 0, 2], sm[:, 1, 2], sm[:, 2, 2] = (NEG * ones, -NEG * sp,
                                             -NEG * (1.0 - sp))   # 2: FRintra
    sm[:, 0, 3], sm[:, 1, 3], sm[:, 2, 3] = ones, u1, u2          # 3: FL dup
    sm[:, 0, 4], sm[:, 1, 4], sm[:, 2, 4] = (NEG * ones,
                                             -NEG * (1.0 - sp) * col,
                                             -NEG * sp * col)     # 4: FRinter
    sm[:, 0, 5] = col                                             # 5
    sm[:, 0, 6] = row                                             # 6
    sm[:, 0, 7] = 1.0 - row                                       # 7
    sm[:, 0, 8] = ones[0]                                         # 8

    import ml_dtypes
    band = (np.abs(j[:, None] - j[None, :]) <= 2)
    g["bandd"] = np.concatenate([band, band],
                                axis=1).astype(ml_dtypes.bfloat16)
    kemo = (emo @ np.asarray(inputs["t_Wk"], f32)
            + np.asarray(inputs["t_bk"], f32))
    g["kTemo"] = np.ascontiguousarray(kemo.T)
    vemo = (emo @ np.asarray(inputs["t_Wv"], f32)
            + np.asarray(inputs["t_bv"], f32))
    vaug = np.ones((E, VW), f32)
    vaug3 = vaug.reshape(E, NH, 128)
    vaug3[:, :, 0:64] = vemo.reshape(E, NH, 64)
    g["vemoaug"] = vaug
    g["tWq"] = np.asarray(inputs["t_Wq"], f32) / np.sqrt(DH).astype(f32)
    g["tWo"] = np.asarray(inputs["t_Wo"], f32)
    g["bWq"] = np.asarray(inputs["b_Wq"], f32) / np.sqrt(DH).astype(f32)
    g["bWk"] = np.asarray(inputs["b_Wk"], f32)
    g["bWv"] = np.asarray(inputs["b_Wv"], f32)
    W1 = np.asarray(inputs["W1"], np.float64)
    bWo = np.asarray(inputs["b_Wo"], np.float64)
    g["What"] = np.stack(
        [(bWo[i] @ W1[i * H:(i + 1) * H]).astype(f32) for i in range(4)])
    ln2g = np.asarray(inputs["ln2_g"], np.float64)
    g["W2p"] = (ln2g[:, None]
                * np.asarray(inputs["W2"], np.float64)).astype(f32)
    g["onesd"] = np.ones(T, f32)
    g["tbq"] = np.asarray(inputs["t_bq"], f32) / np.sqrt(DH).astype(f32)
    g["tbo"] = (np.asarray(inputs["t_bo"], np.float64)
                + np.asarray(inputs["t_bv"], np.float64)
                @ np.asarray(inputs["t_Wo"], np.float64)).astype(f32)
    g["g1"] = np.asarray(inputs["t_ln_g"], f32)
    g["b1v"] = np.asarray(inputs["t_ln_b"], f32)
    g["bbq"] = np.asarray(inputs["b_bq"], f32) / np.sqrt(DH).astype(f32)
    g["bbk"] = np.asarray(inputs["b_bk"], f32)
    bhat = np.asarray(inputs["b1"], np.float64).copy()
    for i in range(4):
        eff = (np.asarray(inputs["b_bo"][i], np.float64)
               + np.asarray(inputs["b_bv"][i], np.float64) @ bWo[i])
        bhat += eff @ W1[i * H:(i + 1) * H]
    g["bhat"] = bhat.astype(f32)
    g["c2row"] = (np.asarray(inputs["ln2_b"], np.float64)
                  @ np.asarray(inputs["W2"], np.float64)).astype(f32)

    apply_g1b1 = not (np.all(inputs["t_ln_g"] == 1.0)
                      and np.all(inputs["t_ln_b"] == 0.0))
    apply_c2 = bool(np.any(g["c2row"] != 0.0))

    in_maps = []
    for c in range(NCORES):
        m = dict(g)
        m["xT"] = np.ascontiguousarray(xTp[c * PR:(c + 1) * PR])
        m["smalls"] = np.ascontiguousarray(sm[c * BL:(c + 1) * BL])
        in_maps.append(m)
    return in_maps, apply_g1b1, apply_c2


def kernel(**inputs):
    in_maps, apply_g1b1, apply_c2 = _host_prep(inputs)
    key = (apply_g1b1, apply_c2)
    if key not in _CACHE:
        _CACHE[key] = _build(*key)
    nc = _CACHE[key]
    res = run_bass_kernel_spmd(nc, in_maps, core_ids=list(range(NCORES)),
                               trace=False)
    outs = [res.results[c]["out"] for c in range(NCORES)]
    return np.concatenate(outs, axis=0)



# revision 10
# speedup vs baseline: 1.1627x; 1.1627x over previous
"""DialogueEIN fused kernel for 8 TRN2 NeuronCores (data-parallel over batch).

Self-contained: hardcodes shapes for the nn_DialogueEIN problem
  x[64,256,512], T=256, H=512, NH=8 heads, E=7 emotion slots, window 5.

v2 strategy (per core, 8 batches as 4 batch-PAIRS, transposed [H, T] space):
  - All 18 H x H projections (t_Wq, t_Wo, branch Wq/Wk/Wv/What) run as
    fp8e4m3 DoubleRow matmuls: K=256 per pass at 0.5 cycles/row -> 4x the
    fp32r projection rate.  W2 and the attention score/PV matmuls stay bf16
    (1 cycle/row).  Predicted end-to-end rel err ~5e-3 (measured in numpy).
  - Softmax without max-subtraction, additive -50 masks accumulated into the
    score PSUM by rank<=3 matmuls (exact in bf16); local branch uses a
    multiplicative post-exp band mask applied on the GPSIMD (Pool) engine.
  - PV lhsT is [V_h | ones64] so PSUM rows 64:128 hold the softmax
    denominator; normalize = DVE reciprocal + multiply-evict (bf16 out).
  - Tendency stage packs 4 heads per score PSUM bank (partition offsets
    0/32/64/96, K rows zero-padded to 32) so exp is 2 ops/pair not 32.
  - LayerNorm over the partition axis via ones-column bf16 matmuls; rstd/mu
    rows broadcast by PE; bf16 2x-rate DVE applies.
  - Host folds: b_Wo[i] @ W1_i, ln2 gamma into W2, t_bv/b_bv into biases,
    1/sqrt(dh) into Wq, biases into eviction activations.
"""

import numpy as np
import ml_dtypes

import concourse.bass as bass
import concourse.mybir as mybir
import concourse.tile as tile
from concourse import bacc
from concourse.bass_utils import run_bass_kernel_spmd

F32 = mybir.dt.float32
F32R = mybir.dt.float32r
BF = mybir.dt.bfloat16
F8 = mybir.dt.float8e4
ALU = mybir.AluOpType
ACTF = mybir.ActivationFunctionType
DR = mybir.MatmulPerfMode.DoubleRow

B, T, H, NH, E = 64, 256, 512, 8, 7
DH = H // NH
NCORES = 8
BL = B // NCORES          # 8 batches per core
PR = BL // 2              # 4 batch-pairs per core
WD = 2 * T                # 512: paired free width
VW = NH * 128             # 1024: V_aug width ([V_h | ones64] per head)
NEG = -50.0
KT = H // 128             # 4
MT = T // 128             # 2
EPS = 1e-12
NW = 18                   # packed DoubleRow weight matrices

_CACHE = {}


def _build(apply_g1b1, apply_c2):
    nc = bacc.Bacc("TRN2", target_bir_lowering=False, debug=False,
                   enable_asserts=False)

    def din(name, shape, dt=F32):
        return nc.dram_tensor(name, list(shape), dt, kind="ExternalInput").ap()

    xT8 = din("xT8", (PR, 2, 128, 2 * WD), F8)
    xTf = din("xTf", (PR, H, WD))
    smalls = din("smalls", (BL, 3, 9, T), BF)
    bandd = din("bandd", (T, WD), BF)
    kTe32 = din("kTe32", (H, 32), BF)
    vea4 = din("vea4", (128, VW), BF)
    w8 = din("w8", (NW, 2, 128, 2 * H), F8)
    W2p = din("W2p", (H, H), BF)
    onesb = din("onesb", (T,), BF)
    tbq = din("tbq", (H,))
    tbo = din("tbo", (H,))
    g1 = din("g1", (H,))
    b1v = din("b1v", (H,))
    bbq = din("bbq", (4, H))
    bbk = din("bbk", (4, H))
    bhat = din("bhat", (H,))
    c2row = din("c2row", (H,))
    out = nc.dram_tensor("out", [BL, T, H], F32, kind="ExternalOutput").ap()

    with nc.allow_low_precision("bf16 pipeline by design"), \
            tile.TileContext(nc) as tc:
        cst = tc.alloc_tile_pool(name="cst", bufs=1)
        per = tc.alloc_tile_pool(name="per", bufs=1)
        wts = tc.alloc_tile_pool(name="wts", bufs=1)
        act = tc.alloc_tile_pool(name="act", bufs=1)
        pmm = tc.alloc_tile_pool(name="pmm", bufs=3, space="PSUM")
        psc = tc.alloc_tile_pool(name="psc", bufs=3, space="PSUM")
        pcx = tc.alloc_tile_pool(name="pcx", bufs=2, space="PSUM")

        # ---- constants ----
        ones128 = cst.tile([128, 1], BF, name="ones128")
        nc.sync.dma_start(out=ones128, in_=onesb[0:128])
        onr32 = cst.tile([1, 128], BF, name="onr32")
        nc.sync.dma_start(out=onr32, in_=onesb[0:128])
        eps_t = cst.tile([1, 1], F32, name="eps_t")
        nc.vector.memset(eps_t, EPS)
        kTe = []
        for k in range(KT):
            t = cst.tile([128, 32], BF, name=f"kTe{k}")
            nc.sync.dma_start(out=t, in_=kTe32[k * 128:(k + 1) * 128, :])
            kTe.append(t)
        vea = cst.tile([128, VW], BF, name="vea4")
        nc.sync.dma_start(out=vea, in_=vea4)
        bandt = []
        for m in range(MT):
            t = cst.tile([128, WD], BF, name=f"band{m}")
            nc.sync.dma_start(out=t, in_=bandd[m * 128:(m + 1) * 128, :])
            bandt.append(t)

        def vec_tiles(v, nm):
            ts = []
            for k in range(KT):
                t = cst.tile([128, 1], F32, name=f"{nm}{k}")
                nc.sync.dma_start(out=t, in_=v[k * 128:(k + 1) * 128])
                ts.append(t)
            return ts

        tbq_t = vec_tiles(tbq, "tbq")
        tbo_t = vec_tiles(tbo, "tbo")
        bhat_t = vec_tiles(bhat, "bhat")
        g1_t = vec_tiles(g1, "g1") if apply_g1b1 else None
        b1_t = vec_tiles(b1v, "b1v") if apply_g1b1 else None
        bbq_t = [vec_tiles(bbq[i], f"bbq{i}") for i in range(4)]
        bbk_t = [vec_tiles(bbk[i], f"bbk{i}") for i in range(4)]
        c2n = None
        if apply_c2:
            c2n = cst.tile([128, H], F32, name="c2n")
            nc.sync.dma_start(
                out=c2n, in_=bass.AP(tensor=c2row.tensor, offset=c2row.offset,
                                     ap=[[0, 128], [1, H]]))

        # persistent per-pair state
        htTb = [[per.tile([128, WD], BF, name=f"htTb_{p}_{k}")
                 for k in range(KT)] for p in range(PR)]
        ht8 = [[per.tile([128, 2 * WD], F8, name=f"ht8_{p}_{b}")
                for b in range(2)] for p in range(PR)]
        ht8v = [[t.rearrange("p (two n) -> p two n", two=2) for t in row]
                for row in ht8]
        h2sb = [[per.tile([128, WD], F32, name=f"h2sb_{p}_{k}")
                 for k in range(KT)] for p in range(PR)]
        # va ring: 2 sets x (bh, mo); ones columns written once
        va_ring = [[per.tile([128, VW], BF, name=f"va_{s}_{j}")
                    for j in range(4)] for s in range(2)]
        for s in range(2):
            for j in range(4):
                v3 = va_ring[s][j].rearrange("p (h d) -> p h d", h=NH)
                nc.gpsimd.memset(v3[:, :, 64:128].bitcast(mybir.dt.uint16),
                                 0x3F80)

        def load_w8(idx, nm):
            ts = []
            for kb in range(2):
                t = wts.tile([128, 2 * H], F8, tag="w", bufs=12, name=nm)
                nc.sync.dma_start(out=t, in_=w8[idx, kb])
                ts.append(t.rearrange("p (two n) -> p two n", two=2))
            return ts

        def proj_dr(wv, rhs8, bias_tiles, tag, evict="act"):
            """[H, WD] = W.T @ rhs (DoubleRow fp8), evict bf16 + bias."""
            res = []
            for mo in range(KT):
                ps = pmm.tile([128, WD], F32, tag="mm", bufs=3, name="psp")
                for kb in range(2):
                    nc.tensor.matmul(ps, wv[kb][:, :, mo * 128:(mo + 1) * 128],
                                     rhs8[kb], start=(kb == 0), stop=(kb == 1),
                                     perf_mode=DR)
                s = act.tile([128, WD], BF, tag=tag, bufs=4, name="proj")
                if evict == "act":
                    nc.scalar.activation(s, ps, ACTF.Identity,
                                         bias=bias_tiles[mo])
                else:
                    nc.vector.tensor_scalar(s, ps, bias_tiles[mo], None,
                                            op0=ALU.add)
                res.append(s)
            return res

        def bcast_row(row_ap):
            t = pcx.tile([128, WD], F32, tag="ctx", bufs=2, name="bcast")
            nc.tensor.matmul(t, onr32, row_ap, start=True, stop=True)
            return t

        def layer_norm_T(s_tiles, gb, dests):
            """LN over the partition (H) axis; s_tiles bf16, dests bf16."""
            psmu = psc.tile([128, WD], F32, tag="sc", bufs=3, name="psmu")
            for k in range(KT):
                nc.tensor.matmul(psmu[0:1, :], ones128, s_tiles[k],
                                 start=(k == 0), stop=(k == KT - 1))
            pss2 = psc.tile([128, WD], F32, tag="sc", bufs=3, name="pss2")
            for k in range(KT):
                sq = act.tile([128, WD], BF, tag="sq", bufs=2, name="sq")
                nc.scalar.activation(sq, s_tiles[k], ACTF.Square)
                nc.tensor.matmul(pss2[0:1, :], ones128, sq,
                                 start=(k == 0), stop=(k == KT - 1))
            mu = act.tile([1, WD], F32, tag="lnstat", bufs=4, name="mu")
            ex2 = act.tile([1, WD], F32, tag="lnstat", bufs=4, name="ex2")
            nc.scalar.activation(mu, psmu[0:1, :], ACTF.Copy, scale=1.0 / H)
            nc.scalar.activation(ex2, pss2[0:1, :], ACTF.Copy, scale=1.0 / H)
            var = act.tile([1, WD], F32, tag="lnv", bufs=3, name="lnv")
            nc.vector.scalar_tensor_tensor(var, mu, -1.0, mu,
                                           op0=ALU.mult, op1=ALU.mult)
            nc.vector.tensor_tensor(out=var, in0=ex2, in1=var, op=ALU.add)
            nc.scalar.activation(var, var, ACTF.Sqrt, bias=eps_t)
            rstd = act.tile([1, WD], BF, tag="lnr", bufs=3, name="lnr")
            nm = act.tile([1, WD], BF, tag="lnr", bufs=3, name="lnn")
            with nc.allow_low_precision("bf16 LN scale rows"):
                nc.vector.reciprocal(rstd, var)
                nc.vector.scalar_tensor_tensor(nm, mu, -1.0, rstd,
                                               op0=ALU.mult, op1=ALU.mult)
            RSp = bcast_row(rstd)
            NMp = bcast_row(nm)
            RS = act.tile([128, WD], BF, tag="lnb", bufs=4, name="RSb")
            NM = act.tile([128, WD], BF, tag="lnb", bufs=4, name="NMb")
            nc.scalar.copy(out=RS, in_=RSp)
            nc.scalar.copy(out=NM, in_=NMp)
            for k in range(KT):
                o = dests[k]
                nc.vector.tensor_tensor(out=o, in0=s_tiles[k], in1=RS,
                                        op=ALU.mult)
                nc.vector.tensor_tensor(out=o, in0=o, in1=NM, op=ALU.add)
                if gb is not None:
                    nc.vector.tensor_scalar(o, o, gb[0][k], gb[1][k],
                                            op0=ALU.mult, op1=ALU.add)
            return dests

        # ---------------- Stage T: tendency attention + LN1 ----------------
        wq_t = load_w8(0, "twq")
        wo_t = load_w8(1, "two")

        for p in range(PR):
            x8 = []
            for kb in range(2):
                t = act.tile([128, 2 * WD], F8, tag="x8", bufs=4, name="x8t")
                nc.sync.dma_start(out=t, in_=xT8[p, kb])
                x8.append(t.rearrange("p (two n) -> p two n", two=2))
            xf = []
            for k in range(KT):
                t = act.tile([128, WD], F32, tag="xT", bufs=8, name="xTt")
                nc.sync.dma_start(out=t, in_=xTf[p, k * 128:(k + 1) * 128, :])
                xf.append(t)
            qT = proj_dr(wq_t, x8, tbq_t, "qT")
            ctxb = [act.tile([128, WD], BF, tag="ctxb", bufs=4, name="ctxb")
                    for _ in range(KT)]
            for g in range(2):
                sps = psc.tile([128, WD], F32, tag="sc", bufs=3, name="scte")
                for hh in range(4):
                    h = 4 * g + hh
                    pi = (h % 2) * 64
                    nc.tensor.matmul(sps[32 * hh:32 * hh + 32, :],
                                     kTe[h // 2][pi:pi + 64, :],
                                     qT[h // 2][pi:pi + 64, :],
                                     start=True, stop=True,
                                     tile_position=(pi, 32 * hh))
                et = act.tile([128, WD], BF, tag="et", bufs=4, name="ett")
                nc.scalar.activation(et, sps, ACTF.Exp)
                for hh in range(4):
                    h = 4 * g + hh
                    po = 32 * hh
                    ps2 = pcx.tile([128, WD], F32, tag="ctx", bufs=2,
                                   name="ctxps")
                    nc.tensor.matmul(ps2, vea[po:po + 7,
                                              h * 128:(h + 1) * 128],
                                     et[po:po + 7, :], start=True, stop=True,
                                     tile_position=(po, 0))
                    recD = act.tile([64, WD], BF, tag="recD", bufs=2,
                                    name="recD")
                    nc.vector.reciprocal(out=recD, in_=ps2[64:128, :])
                    pi = (h % 2) * 64
                    nc.vector.tensor_tensor(out=ctxb[h // 2][pi:pi + 64, :],
                                            in0=ps2[0:64, :], in1=recD,
                                            op=ALU.mult)
            ct8 = []
            for b in range(2):
                t = act.tile([128, 2 * WD], F8, tag="ct8", bufs=2, name="ct8")
                nc.gpsimd.tensor_copy(out=t[:, 0:WD], in_=ctxb[2 * b])
                nc.gpsimd.tensor_copy(out=t[:, WD:2 * WD], in_=ctxb[2 * b + 1])
                ct8.append(t.rearrange("p (two n) -> p two n", two=2))
            s1 = []
            for mo in range(KT):
                ps = pmm.tile([128, WD], F32, tag="mm", bufs=3, name="psh")
                for kb in range(2):
                    nc.tensor.matmul(ps, wo_t[kb][:, :, mo * 128:(mo + 1) * 128],
                                     ct8[kb], start=(kb == 0), stop=(kb == 1),
                                     perf_mode=DR)
                s = act.tile([128, WD], BF, tag="s1", bufs=8, name="s1")
                nc.vector.scalar_tensor_tensor(s, ps, tbo_t[mo], xf[mo],
                                               op0=ALU.add, op1=ALU.add)
                s1.append(s)
            layer_norm_T(s1, (g1_t, b1_t) if apply_g1b1 else None,
                         dests=htTb[p])
            for b in range(2):
                nc.gpsimd.tensor_copy(out=ht8[p][b][:, 0:WD],
                                      in_=htTb[p][2 * b])
                nc.gpsimd.tensor_copy(out=ht8[p][b][:, WD:2 * WD],
                                      in_=htTb[p][2 * b + 1])

        # ---------------- Branch stages ----------------
        for i in range(4):
            wq_b = load_w8(2 + 4 * i, "bwq")
            wk_b = load_w8(3 + 4 * i, "bwk")
            wv_b = load_w8(4 + 4 * i, "bwv")
            wh_b = load_w8(5 + 4 * i, "bwh")
            gsl = {0: slice(0, 2), 1: slice(5, 9),
                   2: slice(1, 3), 3: slice(3, 5)}[i]
            ng = gsl.stop - gsl.start
            for p in range(PR):
                sms = []
                for bh in range(2):
                    sm = act.tile([3, 4, T], BF, tag="sm", bufs=2, name="sm")
                    sm = sm[:, 0:ng, :]
                    nc.sync.dma_start(out=sm,
                                      in_=smalls[2 * p + bh][:, gsl, :])
                    sms.append(sm)
                ml = None
                if i == 1:  # local: multiplicative mask band*outer(col,row)+B
                    ml = []
                    for m in range(MT):
                        msl = slice(m * 128, (m + 1) * 128)
                        psA = psc.tile([128, WD], F32, tag="sc", bufs=3,
                                       name="psA")
                        psB = psc.tile([128, WD], F32, tag="sc", bufs=3,
                                       name="psB")
                        for bh in range(2):
                            hsl = slice(bh * T, (bh + 1) * T)
                            nc.tensor.matmul(psA[:, hsl], sms[bh][0:1, 0, msl],
                                             sms[bh][0:1, 1, :],
                                             start=True, stop=True)
                            nc.tensor.matmul(psB[:, hsl], sms[bh][0:1, 3, msl],
                                             sms[bh][0:1, 2, :],
                                             start=True, stop=True)
                        mt_ = act.tile([128, WD], BF, tag="ml", bufs=2,
                                       name="ml")
                        nc.vector.tensor_tensor(out=mt_, in0=psA,
                                                in1=bandt[m], op=ALU.mult)
                        nc.vector.tensor_tensor(out=mt_, in0=mt_, in1=psB,
                                                op=ALU.add)
                        ml.append(mt_)
                qT = proj_dr(wq_b, ht8v[p], bbq_t[i], "qT")
                kT = proj_dr(wk_b, ht8v[p], bbk_t[i], "kT", evict="dve")
                vs = va_ring[(i * PR + p) % 2]
                va = [[None] * MT for _ in range(2)]
                for bh in range(2):
                    for mo in range(MT):
                        ps = pmm.tile([128, H], F32, tag="mm", bufs=3,
                                      name="psv")
                        for kb in range(2):
                            off = bh * T + mo * 128
                            nc.tensor.matmul(
                                ps, ht8v[p][kb][:, :, off:off + 128],
                                wv_b[kb], start=(kb == 0), stop=(kb == 1),
                                perf_mode=DR)
                        vt = vs[2 * bh + mo]
                        vt3 = vt.rearrange("p (h d) -> p h d", h=NH)
                        ps3 = ps.rearrange("p (h d) -> p h d", h=NH)
                        nc.scalar.activation(vt3[:, :, 0:64], ps3, ACTF.Copy)
                        va[bh][mo] = vt
                ctxb = [act.tile([128, WD], BF, tag="ctxb", bufs=4,
                                 name="ctxb") for _ in range(KT)]
                for h in range(NH):
                    pi = (h % 2) * 64
                    ets = []
                    for m in range(MT):
                        msl = slice(m * 128, (m + 1) * 128)
                        sps = psc.tile([128, WD], F32, tag="sc", bufs=3,
                                       name="scb")
                        for bh in range(2):
                            hsl = slice(bh * T, (bh + 1) * T)
                            nc.tensor.matmul(
                                sps[:, hsl],
                                kT[h // 2][pi:pi + 64, bh * T + m * 128:
                                           bh * T + m * 128 + 128],
                                qT[h // 2][pi:pi + 64, hsl],
                                start=True, stop=(i == 1))
                            if i == 0:
                                nc.tensor.matmul(sps[:, hsl],
                                                 sms[bh][0:1, 0, msl],
                                                 sms[bh][0:1, 1, :],
                                                 start=False, stop=True)
                            elif i in (2, 3):
                                nc.tensor.matmul(sps[:, hsl],
                                                 sms[bh][:, 1, msl],
                                                 sms[bh][:, 0, :],
                                                 start=False, stop=True)
                        et = act.tile([128, WD], BF, tag="et", bufs=4,
                                      name="etb")
                        nc.scalar.activation(et, sps, ACTF.Exp)
                        if i == 1:
                            nc.gpsimd.tensor_tensor(out=et, in0=et, in1=ml[m],
                                                    op=ALU.mult)
                        ets.append(et)
                    ps2 = pcx.tile([128, WD], F32, tag="ctx", bufs=2,
                                   name="ctxps")
                    for bh in range(2):
                        hsl = slice(bh * T, (bh + 1) * T)
                        for kt in range(MT):
                            nc.tensor.matmul(ps2[:, hsl],
                                             va[bh][kt][:, h * 128:
                                                        (h + 1) * 128],
                                             ets[kt][:, hsl], start=(kt == 0),
                                             stop=(kt == MT - 1))
                    recD = act.tile([64, WD], BF, tag="recD", bufs=2,
                                    name="recD")
                    nc.vector.reciprocal(out=recD, in_=ps2[64:128, :])
                    nc.vector.tensor_tensor(out=ctxb[h // 2][pi:pi + 64, :],
                                            in0=ps2[0:64, :], in1=recD,
                                            op=ALU.mult)
                ct8 = []
                for b in range(2):
                    t = act.tile([128, 2 * WD], F8, tag="ct8", bufs=2,
                                 name="ct8")
                    nc.gpsimd.tensor_copy(out=t[:, 0:WD], in_=ctxb[2 * b])
                    nc.gpsimd.tensor_copy(out=t[:, WD:2 * WD],
                                          in_=ctxb[2 * b + 1])
                    ct8.append(t.rearrange("p (two n) -> p two n", two=2))
                for mo in range(KT):
                    ps = pmm.tile([128, WD], F32, tag="mm", bufs=3,
                                  name="psh2")
                    for kb in range(2):
                        nc.tensor.matmul(
                            ps, wh_b[kb][:, :, mo * 128:(mo + 1) * 128],
                            ct8[kb], start=(kb == 0), stop=(kb == 1),
                            perf_mode=DR)
                    if i == 0:
                        nc.scalar.copy(out=h2sb[p][mo], in_=ps)
                    elif i < 3:
                        nc.vector.tensor_tensor(out=h2sb[p][mo],
                                                in0=h2sb[p][mo], in1=ps,
                                                op=ALU.add)
                    else:
                        nc.vector.scalar_tensor_tensor(
                            h2sb[p][mo], ps, bhat_t[mo], h2sb[p][mo],
                            op0=ALU.add, op1=ALU.add)

        # ---------------- Final: residual + LN2 + W2 ----------------
        w2_t = [wts.tile([128, H], BF, tag="w", bufs=12, name="w2p")
                for _ in range(KT)]
        for k in range(KT):
            nc.sync.dma_start(out=w2_t[k], in_=W2p[k * 128:(k + 1) * 128, :])
        for p in range(PR):
            s2 = []
            for k in range(KT):
                s = act.tile([128, WD], BF, tag="s2", bufs=8, name="s2")
                nc.vector.tensor_tensor(out=s, in0=h2sb[p][k],
                                        in1=htTb[p][k], op=ALU.add)
                s2.append(s)
            n2 = [act.tile([128, WD], BF, tag="n2", bufs=8, name="n2")
                  for _ in range(KT)]
            layer_norm_T(s2, None, dests=n2)
            for bh in range(2):
                for mo in range(MT):
                    ps = pmm.tile([128, H], F32, tag="mm", bufs=3, name="pso")
                    for ki in range(KT):
                        off = bh * T + mo * 128
                        nc.tensor.matmul(ps, n2[ki][:, off:off + 128],
                                         w2_t[ki], start=(ki == 0),
                                         stop=(ki == KT - 1))
                    osb = act.tile([128, H], F32, tag="osb", bufs=2,
                                   name="osb")
                    if apply_c2:
                        nc.vector.tensor_tensor(out=osb, in0=ps, in1=c2n,
                                                op=ALU.add)
                    else:
                        nc.scalar.copy(out=osb, in_=ps)
                    nc.sync.dma_start(
                        out=out[2 * p + bh, mo * 128:(mo + 1) * 128, :],
                        in_=osb)
        pcx.release()
        psc.release()
        pmm.release()
        act.release()
        wts.release()
        per.release()
        cst.release()

    nc.compile()
    return nc


def _host_prep(inputs):
    f32 = np.float32
    E4 = ml_dtypes.float8_e4m3
    BF16 = ml_dtypes.bfloat16
    g = {}
    x = np.asarray(inputs["x"], f32)
    lengths = np.asarray(inputs["lengths"])
    speakers = np.asarray(inputs["speakers"])
    emo = np.asarray(inputs["emo_table"], f32)

    xTa = np.ascontiguousarray(x.transpose(0, 2, 1))  # [B, H, T]
    xTp = np.ascontiguousarray(
        xTa.reshape(B // 2, 2, H, T).transpose(0, 2, 1, 3).reshape(
            B // 2, H, WD))
    # fp8 DoubleRow rhs layout: [pr, kb, p, i*WD + j] = xTp[pr, 256kb+128i+p, j]
    xT8 = np.ascontiguousarray(
        xTp.astype(E4).reshape(B // 2, 2, 2, 128, WD).transpose(
            0, 1, 3, 2, 4).reshape(B // 2, 2, 128, 2 * WD))

    j = np.arange(T)
    row = (j[None, :] < lengths[:, None]).astype(f32)
    col = row
    sp = speakers.astype(f32)
    u1 = row * sp
    u2 = row * (1.0 - sp)
    ones = np.ones_like(row)
    sm = np.zeros((B, 3, 9, T), f32)
    sm[:, 0, 0] = NEG * (1.0 - col)                               # 0: FR glob
    sm[:, 0, 1], sm[:, 1, 1], sm[:, 2, 1] = ones, u1, u2          # 1: FL
    sm[:, 0, 2], sm[:, 1, 2], sm[:, 2, 2] = (NEG * ones, -NEG * sp,
                                             -NEG * (1.0 - sp))   # 2: FRintra
    sm[:, 0, 3], sm[:, 1, 3], sm[:, 2, 3] = ones, u1, u2          # 3: FL dup
    sm[:, 0, 4], sm[:, 1, 4], sm[:, 2, 4] = (NEG * ones,
                                             -NEG * (1.0 - sp) * col,
                                             -NEG * sp * col)     # 4: FRinter
    sm[:, 0, 5] = col                                             # 5
    sm[:, 0, 6] = row                                             # 6
    sm[:, 0, 7] = 1.0 - row                                       # 7
    sm[:, 0, 8] = ones[0]                                         # 8
    sm_b = sm.astype(BF16)

    band = (np.abs(j[:, None] - j[None, :]) <= 2)
    g["bandd"] = np.concatenate([band, band], axis=1).astype(BF16)
    kemo = (emo @ np.asarray(inputs["t_Wk"], f32)
            + np.asarray(inputs["t_bk"], f32))
    kTe32 = np.zeros((H, 32), f32)
    kTe32[:, 0:E] = kemo.T
    g["kTe32"] = kTe32.astype(BF16)
    vemo = (emo @ np.asarray(inputs["t_Wv"], f32)
            + np.asarray(inputs["t_bv"], f32))
    vaug = np.ones((E, VW), f32)
    vaug3 = vaug.reshape(E, NH, 128)
    vaug3[:, :, 0:64] = vemo.reshape(E, NH, 64)
    vea4 = np.zeros((128, VW), f32)
    for gg in range(4):
        vea4[32 * gg:32 * gg + E] = vaug
    g["vea4"] = vea4.astype(BF16)

    def pack_dr(W):
        A = np.asarray(W, f32).astype(E4)
        return A.reshape(2, 2, 128, H).transpose(0, 2, 1, 3).reshape(
            2, 128, 2 * H)

    isq = 1.0 / np.sqrt(DH)
    W1 = np.asarray(inputs["W1"], np.float64)
    bWo = np.asarray(inputs["b_Wo"], np.float64)
    What = [(bWo[i] @ W1[i * H:(i + 1) * H]).astype(f32) for i in range(4)]
    mats = [np.asarray(inputs["t_Wq"], f32) * isq,
            np.asarray(inputs["t_Wo"], f32)]
    for i in range(4):
        mats += [np.asarray(inputs["b_Wq"][i], f32) * isq,
                 np.asarray(inputs["b_Wk"][i], f32),
                 np.asarray(inputs["b_Wv"][i], f32),
                 What[i]]
    g["w8"] = np.stack([pack_dr(m) for m in mats])

    ln2g = np.asarray(inputs["ln2_g"], np.float64)
    g["W2p"] = (ln2g[:, None]
                * np.asarray(inputs["W2"], np.float64)).astype(BF16)
    g["onesb"] = np.ones(T, BF16)
    g["tbq"] = np.asarray(inputs["t_bq"], f32) * isq
    g["tbo"] = (np.asarray(inputs["t_bo"], np.float64)
                + np.asarray(inputs["t_bv"], np.float64)
                @ np.asarray(inputs["t_Wo"], np.float64)).astype(f32)
    g["g1"] = np.asarray(inputs["t_ln_g"], f32)
    g["b1v"] = np.asarray(inputs["t_ln_b"], f32)
    g["bbq"] = np.asarray(inputs["b_bq"], f32) * isq
    g["bbk"] = np.asarray(inputs["b_bk"], f32)
    bhat = np.asarray(inputs["b1"], np.float64).copy()
    for i in range(4):
        eff = (np.asarray(inputs["b_bo"][i], np.float64)
               + np.asarray(inputs["b_bv"][i], np.float64) @ bWo[i])
        bhat += eff @ W1[i * H:(i + 1) * H]
    g["bhat"] = bhat.astype(f32)
    g["c2row"] = (np.asarray(inputs["ln2_b"], np.float64)
                  @ np.asarray(inputs["W2"], np.float64)).astype(f32)

    apply_g1b1 = not (np.all(inputs["t_ln_g"] == 1.0)
                      and np.all(inputs["t_ln_b"] == 0.0))
    apply_c2 = bool(np.any(g["c2row"] != 0.0))

    in_maps = []
    for c in range(NCORES):
        m = dict(g)
        m["xT8"] = np.ascontiguousarray(xT8[c * PR:(c + 1) * PR])
        m["xTf"] = np.ascontiguousarray(xTp[c * PR:(c + 1) * PR])
        m["smalls"] = np.ascontiguousarray(sm_b[c * BL:(c + 1) * BL])
        in_maps.append(m)
    return in_maps, apply_g1b1, apply_c2


def kernel(**inputs):
    in_maps, apply_g1b1, apply_c2 = _host_prep(inputs)
    key = (apply_g1b1, apply_c2)
    if key not in _CACHE:
        _CACHE[key] = _build(*key)
    nc = _CACHE[key]
    res = run_bass_kernel_spmd(nc, in_maps, core_ids=list(range(NCORES)),
                               trace=False)
    outs = [res.results[c]["out"] for c in range(NCORES)]
    return np.concatenate(outs, axis=0)


# revision 32
# speedup vs baseline: 1.1786x; 1.0137x over previous
"""DialogueEIN fused kernel for 8 TRN2 NeuronCores (data-parallel over batch).

Self-contained: hardcodes shapes for the nn_DialogueEIN problem
  x[64,256,512], T=256, H=512, NH=8 heads, E=7 emotion slots, window 5.

v2 strategy (per core, 8 batches as 4 batch-PAIRS, transposed [H, T] space):
  - All 18 H x H projections (t_Wq, t_Wo, branch Wq/Wk/Wv/What) run as
    fp8e4m3 DoubleRow matmuls: K=256 per pass at 0.5 cycles/row -> 4x the
    fp32r projection rate.  W2 and the attention score/PV matmuls stay bf16
    (1 cycle/row).  Predicted end-to-end rel err ~5e-3 (measured in numpy).
  - Softmax without max-subtraction, additive -50 masks accumulated into the
    score PSUM by rank<=3 matmuls (exact in bf16); local branch uses a
    multiplicative post-exp band mask applied on the GPSIMD (Pool) engine.
  - PV lhsT is [V_h | ones64] so PSUM rows 64:128 hold the softmax
    denominator; normalize = DVE reciprocal + multiply-evict (bf16 out).
  - Tendency stage packs 4 heads per score PSUM bank (partition offsets
    0/32/64/96, K rows zero-padded to 32) so exp is 2 ops/pair not 32.
  - LayerNorm over the partition axis via ones-column bf16 matmuls; rstd/mu
    rows broadcast by PE; bf16 2x-rate DVE applies.
  - Host folds: b_Wo[i] @ W1_i, ln2 gamma into W2, t_bv/b_bv into biases,
    1/sqrt(dh) into Wq, biases into eviction activations.
"""

import numpy as np
import ml_dtypes

import concourse.bass as bass
import concourse.mybir as mybir
import concourse.tile as tile
from concourse import bacc
from concourse.bass_utils import run_bass_kernel_spmd

F32 = mybir.dt.float32
F32R = mybir.dt.float32r
BF = mybir.dt.bfloat16
F8 = mybir.dt.float8e4
ALU = mybir.AluOpType
ACTF = mybir.ActivationFunctionType
DR = mybir.MatmulPerfMode.DoubleRow

B, T, H, NH, E = 64, 256, 512, 8, 7
DH = H // NH
NCORES = 8
BL = B // NCORES          # 8 batches per core
PR = BL // 2              # 4 batch-pairs per core
WD = 2 * T                # 512: paired free width
VW = NH * 128             # 1024: V_aug width ([V_h | ones64] per head)
NEG = -50.0
KT = H // 128             # 4
MT = T // 128             # 2
EPS = 1e-12
NW = 18                   # packed DoubleRow weight matrices

_CACHE = {}


def _build(apply_g1b1, apply_c2):
    nc = bacc.Bacc("TRN2", target_bir_lowering=False, debug=False,
                   enable_asserts=False)

    def din(name, shape, dt=F32):
        return nc.dram_tensor(name, list(shape), dt, kind="ExternalInput").ap()

    xT8 = din("xT8", (PR, 2, 128, 2 * WD), F8)
    xTf = din("xTf", (PR, H, WD), BF)
    Lall = din("Lall", (PR, 2, 128, T), BF)
    Rall = din("Rall", (PR, 128, WD), BF)
    bandd = din("bandd", (T, WD), BF)
    kTe32 = din("kTe32", (H, 32), BF)
    vea4 = din("vea4", (128, VW), BF)
    w8 = din("w8", (NW, 2, 128, 2 * H), F8)
    W2p = din("W2p", (H, H), BF)
    onesb = din("onesb", (T,), BF)
    tbq = din("tbq", (H,))
    tbo = din("tbo", (H,))
    g1 = din("g1", (H,))
    b1v = din("b1v", (H,))
    bbq = din("bbq", (4, H))
    bbk = din("bbk", (4, H))
    bhat = din("bhat", (H,))
    c2row = din("c2row", (H,))
    out = nc.dram_tensor("out", [BL, T, H], F32, kind="ExternalOutput").ap()

    with nc.allow_low_precision("bf16 pipeline by design"), \
            tile.TileContext(nc) as tc:
        cst = tc.alloc_tile_pool(name="cst", bufs=1)
        per = tc.alloc_tile_pool(name="per", bufs=1)
        wts = tc.alloc_tile_pool(name="wts", bufs=1)
        act = tc.alloc_tile_pool(name="act", bufs=1)
        pmm = tc.alloc_tile_pool(name="pmm", bufs=3, space="PSUM")
        psc = tc.alloc_tile_pool(name="psc", bufs=3, space="PSUM")
        pcx = tc.alloc_tile_pool(name="pcx", bufs=2, space="PSUM")

        # ---- constants ----
        ones128 = cst.tile([128, 1], BF, name="ones128")
        nc.sync.dma_start(out=ones128, in_=onesb[0:128])
        onr32 = cst.tile([1, 128], BF, name="onr32")
        nc.sync.dma_start(out=onr32, in_=onesb[0:128])
        eps_t = cst.tile([1, 1], F32, name="eps_t")
        nc.vector.memset(eps_t, EPS)
        kTe = []
        for k in range(KT):
            t = cst.tile([128, 32], BF, name=f"kTe{k}")
            nc.sync.dma_start(out=t, in_=kTe32[k * 128:(k + 1) * 128, :])
            kTe.append(t)
        vea = cst.tile([128, VW], BF, name="vea4")
        nc.sync.dma_start(out=vea, in_=vea4)
        bandt = []
        for m in range(MT):
            t = cst.tile([128, WD], BF, name=f"band{m}")
            nc.sync.dma_start(out=t, in_=bandd[m * 128:(m + 1) * 128, :])
            bandt.append(t)

        def vec_tiles(v, nm):
            ts = []
            for k in range(KT):
                t = cst.tile([128, 1], F32, name=f"{nm}{k}")
                nc.sync.dma_start(out=t, in_=v[k * 128:(k + 1) * 128])
                ts.append(t)
            return ts

        tbq_t = vec_tiles(tbq, "tbq")
        tbo_t = vec_tiles(tbo, "tbo")
        bhat_t = vec_tiles(bhat, "bhat")
        g1_t = vec_tiles(g1, "g1") if apply_g1b1 else None
        b1_t = vec_tiles(b1v, "b1v") if apply_g1b1 else None
        bbq_t = [vec_tiles(bbq[i], f"bbq{i}") for i in range(4)]
        bbk_t = [vec_tiles(bbk[i], f"bbk{i}") for i in range(4)]
        c2n = None
        if apply_c2:
            c2n = cst.tile([128, H], F32, name="c2n")
            nc.sync.dma_start(
                out=c2n, in_=bass.AP(tensor=c2row.tensor, offset=c2row.offset,
                                     ap=[[0, 128], [1, H]]))

        # persistent per-pair state
        htTb = [[per.tile([128, WD], BF, name=f"htTb_{p}_{k}")
                 for k in range(KT)] for p in range(PR)]
        ht8 = [[per.tile([128, 2 * WD], F8, name=f"ht8_{p}_{b}")
                for b in range(2)] for p in range(PR)]
        ht8v = [[t.rearrange("p (two n) -> p two n", two=2) for t in row]
                for row in ht8]
        h2sb = [[per.tile([128, WD], F32, name=f"h2sb_{p}_{k}")
                 for k in range(KT)] for p in range(PR)]
        # va ring: 2 sets x (bh, mo); ones columns written once
        va_ring = [[per.tile([128, VW], BF, name=f"va_{s}_{j}")
                    for j in range(4)] for s in range(2)]
        for s in range(2):
            for j in range(4):
                v3 = va_ring[s][j].rearrange("p (h d) -> p h d", h=NH)
                nc.gpsimd.memset(v3[:, :, 64:128].bitcast(mybir.dt.uint16),
                                 0x3F80)

        def load_w8(idx, nm):
            ts = []
            for kb in range(2):
                t = wts.tile([128, 2 * H], F8, tag="wt", bufs=4, name=nm)
                nc.sync.dma_start(out=t, in_=w8[idx, kb])
                ts.append(t.rearrange("p (two n) -> p two n", two=2))
            return ts

        # per-pair combined mask factors (keys x T, query-factors x WD),
        # groups at 32-aligned partition offsets:
        #   L1: glob@0(2) locA@32(2) locB@64(2) intra@96(6); L2: inter@96(6)
        #   R : globq@0   locAq@32   locBq@64   trio@96 (intra+inter share)
        Lt, L2t, Rt = [], [], []
        for p in range(PR):
            lt = per.tile([128, T], BF, name=f"Lt{p}")
            nc.sync.dma_start(out=lt, in_=Lall[p, 0])
            l2 = per.tile([128, T], BF, name=f"L2t{p}")
            nc.sync.dma_start(out=l2, in_=Lall[p, 1])
            rt = per.tile([128, WD], BF, name=f"Rt{p}")
            nc.sync.dma_start(out=rt, in_=Rall[p])
            Lt.append(lt)
            L2t.append(l2)
            Rt.append(rt)

        def proj_dr(wv, rhs8, bias_tiles, tag, evict="act"):
            """[H, WD] = W.T @ rhs (DoubleRow fp8), evict bf16 + bias."""
            res = []
            for mo in range(KT):
                ps = pmm.tile([128, WD], F32, tag="mm", bufs=3, name="psp")
                for kb in range(2):
                    nc.tensor.matmul(ps, wv[kb][:, :, mo * 128:(mo + 1) * 128],
                                     rhs8[kb], start=(kb == 0), stop=(kb == 1),
                                     perf_mode=DR)
                s = act.tile([128, WD], BF, tag=tag, bufs=8, name="proj")
                if evict == "act":
                    nc.scalar.activation(s, ps, ACTF.Identity,
                                         bias=bias_tiles[mo])
                else:
                    nc.vector.tensor_scalar(s, ps, bias_tiles[mo], None,
                                            op0=ALU.add)
                res.append(s)
            return res

        def bcast_row(row_ap):
            t = pcx.tile([128, WD], F32, tag="ctx", bufs=2, name="bcast")
            nc.tensor.matmul(t, onr32, row_ap, start=True, stop=True)
            return t

        def layer_norm_T(s_tiles, gb, dests):
            """LN over the partition (H) axis; s_tiles bf16, dests bf16."""
            psmu = psc.tile([128, WD], F32, tag="sc", bufs=3, name="psmu")
            for k in range(KT):
                nc.tensor.matmul(psmu[0:1, :], ones128, s_tiles[k],
                                 start=(k == 0), stop=(k == KT - 1))
            pss2 = psc.tile([128, WD], F32, tag="sc", bufs=3, name="pss2")
            for k in range(KT):
                sq = act.tile([128, WD], BF, tag="sq", bufs=2, name="sq")
                nc.scalar.activation(sq, s_tiles[k], ACTF.Square)
                nc.tensor.matmul(pss2[0:1, :], ones128, sq,
                                 start=(k == 0), stop=(k == KT - 1))
            mu = act.tile([1, WD], F32, tag="lnstat", bufs=2, name="mu")
            ex2 = act.tile([1, WD], F32, tag="lnstat", bufs=2, name="ex2")
            nc.scalar.activation(mu, psmu[0:1, :], ACTF.Copy, scale=1.0 / H)
            nc.scalar.activation(ex2, pss2[0:1, :], ACTF.Copy, scale=1.0 / H)
            var = act.tile([1, WD], F32, tag="lnv", bufs=2, name="lnv")
            nc.vector.scalar_tensor_tensor(var, mu, -1.0, mu,
                                           op0=ALU.mult, op1=ALU.mult)
            nc.vector.tensor_tensor(out=var, in0=ex2, in1=var, op=ALU.add)
            nc.scalar.activation(var, var, ACTF.Sqrt, bias=eps_t)
            rstd = act.tile([1, WD], BF, tag="lnr", bufs=3, name="lnr")
            nm = act.tile([1, WD], BF, tag="lnr", bufs=3, name="lnn")
            with nc.allow_low_precision("bf16 LN scale rows"):
                nc.vector.reciprocal(rstd, var)
                nc.vector.scalar_tensor_tensor(nm, mu, -1.0, rstd,
                                               op0=ALU.mult, op1=ALU.mult)
            RSp = bcast_row(rstd)
            NMp = bcast_row(nm)
            RS = act.tile([128, WD], BF, tag="lnb", bufs=4, name="RSb")
            NM = act.tile([128, WD], BF, tag="lnb", bufs=4, name="NMb")
            nc.scalar.copy(out=RS, in_=RSp)
            nc.scalar.copy(out=NM, in_=NMp)
            for k in range(KT):
                o = dests[k]
                nc.vector.tensor_tensor(out=o, in0=s_tiles[k], in1=RS,
                                        op=ALU.mult)
                nc.vector.tensor_tensor(out=o, in0=o, in1=NM, op=ALU.add)
                if gb is not None:
                    nc.vector.tensor_scalar(o, o, gb[0][k], gb[1][k],
                                            op0=ALU.mult, op1=ALU.add)
            return dests

        # ---------------- Stage T: tendency attention + LN1 ----------------
        wq_t = load_w8(0, "twq")
        wo_t = load_w8(1, "two")

        for p in range(PR):
            x8 = []
            for kb in range(2):
                t = act.tile([128, 2 * WD], F8, tag="x8", bufs=4, name="x8t")
                nc.sync.dma_start(out=t, in_=xT8[p, kb])
                x8.append(t.rearrange("p (two n) -> p two n", two=2))
            xf = []
            for k in range(KT):
                t = act.tile([128, WD], BF, tag="xT", bufs=8, name="xTt")
                nc.sync.dma_start(out=t, in_=xTf[p, k * 128:(k + 1) * 128, :])
                xf.append(t)
            qT = proj_dr(wq_t, x8, tbq_t, "qT")
            ct8r = [act.tile([128, 2 * WD], F8, tag="ct8", bufs=4, name="ct8")
                    for _ in range(2)]
            for g in range(2):
                sps = psc.tile([128, WD], F32, tag="sc", bufs=3, name="scte")
                for hh in range(4):
                    h = 4 * g + hh
                    pi = (h % 2) * 64
                    nc.tensor.matmul(sps[32 * hh:32 * hh + 32, :],
                                     kTe[h // 2][pi:pi + 64, :],
                                     qT[h // 2][pi:pi + 64, :],
                                     start=True, stop=True,
                                     tile_position=(pi, 32 * hh))
                et = act.tile([128, WD], BF, tag="et", bufs=8, name="ett")
                nc.scalar.activation(et, sps, ACTF.Exp)
                for hh in range(4):
                    h = 4 * g + hh
                    po = 32 * hh
                    ps2 = pcx.tile([128, WD], F32, tag="ctx", bufs=2,
                                   name="ctxps")
                    nc.tensor.matmul(ps2, vea[po:po + 7,
                                              h * 128:(h + 1) * 128],
                                     et[po:po + 7, :], start=True, stop=True,
                                     tile_position=(po, 0))
                    recD = act.tile([64, WD], BF, tag="recD", bufs=4,
                                    name="recD")
                    nc.vector.reciprocal(out=recD, in_=ps2[64:128, :])
                    pi = (h % 2) * 64
                    hf = ((h // 2) % 2) * WD
                    nc.vector.tensor_tensor(
                        out=ct8r[h // 4][pi:pi + 64, hf:hf + WD],
                        in0=ps2[0:64, :], in1=recD, op=ALU.mult)
            ct8 = [t.rearrange("p (two n) -> p two n", two=2) for t in ct8r]
            s1 = []
            for mo in range(KT):
                ps = pmm.tile([128, WD], F32, tag="mm", bufs=3, name="psh")
                for kb in range(2):
                    nc.tensor.matmul(ps, wo_t[kb][:, :, mo * 128:(mo + 1) * 128],
                                     ct8[kb], start=(kb == 0), stop=(kb == 1),
                                     perf_mode=DR)
                s = act.tile([128, WD], BF, tag="s1", bufs=6, name="s1")
                nc.vector.scalar_tensor_tensor(s, ps, tbo_t[mo], xf[mo],
                                               op0=ALU.add, op1=ALU.add)
                s1.append(s)
            layer_norm_T(s1, (g1_t, b1_t) if apply_g1b1 else None,
                         dests=htTb[p])
            for b in range(2):
                nc.gpsimd.tensor_copy(out=ht8[p][b][:, 0:WD],
                                      in_=htTb[p][2 * b])
                nc.gpsimd.tensor_copy(out=ht8[p][b][:, WD:2 * WD],
                                      in_=htTb[p][2 * b + 1])

        # ---------------- Branch stages ----------------
        for i in range(4):
            wall = wts.tile([128, 4 * 2 * 2 * H], F8, tag="wall", bufs=2,
                            name="wall")
            nc.sync.dma_start(
                out=wall,
                in_=bass.AP(tensor=w8.tensor,
                            offset=w8.offset + (2 + 4 * i) * 2 * 128 * 2 * H,
                            ap=[[2 * H, 128], [2 * 128 * 2 * H, 4],
                                [128 * 2 * H, 2], [1, 2 * H]]))
            w5 = wall.rearrange("p (w kb two n) -> p w kb two n", w=4, kb=2,
                                two=2)
            wq_b = [w5[:, 0, kb] for kb in range(2)]
            wk_b = [w5[:, 1, kb] for kb in range(2)]
            wv_b = [w5[:, 2, kb] for kb in range(2)]
            wh_b = [w5[:, 3, kb] for kb in range(2)]
            for p in range(PR):
                ml = None
                if i == 1:  # local: multiplicative mask band*outer(col,row)+B
                    ml = []
                    for m in range(MT):
                        msl = slice(m * 128, (m + 1) * 128)
                        psA = psc.tile([128, WD], F32, tag="sc", bufs=3,
                                       name="psA")
                        psB = psc.tile([128, WD], F32, tag="sc", bufs=3,
                                       name="psB")
                        nc.tensor.matmul(psA, Lt[p][32:34, msl],
                                         Rt[p][32:34, :], start=True,
                                         stop=True, tile_position=(32, 0))
                        nc.tensor.matmul(psB, Lt[p][64:66, msl],
                                         Rt[p][64:66, :], start=True,
                                         stop=True, tile_position=(64, 0))
                        mt_ = act.tile([128, WD], BF, tag="ml", bufs=2,
                                       name="ml")
                        nc.vector.tensor_tensor(out=mt_, in0=psA,
                                                in1=bandt[m], op=ALU.mult)
                        nc.vector.tensor_tensor(out=mt_, in0=mt_, in1=psB,
                                                op=ALU.add)
                        ml.append(mt_)
                qT = proj_dr(wq_b, ht8v[p], bbq_t[i], "qT")
                kT = proj_dr(wk_b, ht8v[p], bbk_t[i], "kT", evict="dve")
                vs = va_ring[(i * PR + p) % 2]
                va = [[None] * MT for _ in range(2)]
                for bh in range(2):
                    for mo in range(MT):
                        ps = pmm.tile([128, H], F32, tag="mm", bufs=3,
                                      name="psv")
                        for kb in range(2):
                            off = bh * T + mo * 128
                            nc.tensor.matmul(
                                ps, ht8v[p][kb][:, :, off:off + 128],
                                wv_b[kb], start=(kb == 0), stop=(kb == 1),
                                perf_mode=DR)
                        vt = vs[2 * bh + mo]
                        vt3 = vt.rearrange("p (h d) -> p h d", h=NH)
                        ps3 = ps.rearrange("p (h d) -> p h d", h=NH)
                        nc.scalar.activation(vt3[:, :, 0:64], ps3, ACTF.Copy)
                        va[bh][mo] = vt
                ct8r = [act.tile([128, 2 * WD], F8, tag="ct8", bufs=4,
                                 name="ct8") for _ in range(2)]
                for h in range(NH):
                    pi = (h % 2) * 64
                    ets = []
                    for m in range(MT):
                        msl = slice(m * 128, (m + 1) * 128)
                        sps = psc.tile([128, WD], F32, tag="sc", bufs=3,
                                       name="scb")
                        for bh in range(2):
                            hsl = slice(bh * T, (bh + 1) * T)
                            nc.tensor.matmul(
                                sps[:, hsl],
                                kT[h // 2][pi:pi + 64, bh * T + m * 128:
                                           bh * T + m * 128 + 128],
                                qT[h // 2][pi:pi + 64, hsl],
                                start=True, stop=(i == 1))
                        if i != 1:
                            lsrc = L2t[p] if i == 3 else Lt[p]
                            po, nr = (0, 2) if i == 0 else (96, 6)
                            nc.tensor.matmul(sps, lsrc[po:po + nr, msl],
                                             Rt[p][po:po + nr, :],
                                             start=False, stop=True,
                                             tile_position=(po, 0))
                        et = act.tile([128, WD], BF, tag="et", bufs=8,
                                      name="etb")
                        nc.scalar.activation(et, sps, ACTF.Exp)
                        if i == 1:
                            nc.gpsimd.tensor_tensor(out=et, in0=et, in1=ml[m],
                                                    op=ALU.mult)
                        ets.append(et)
                    ps2 = pcx.tile([128, WD], F32, tag="ctx", bufs=2,
                                   name="ctxps")
                    for bh in range(2):
                        hsl = slice(bh * T, (bh + 1) * T)
                        for kt in range(MT):
                            nc.tensor.matmul(ps2[:, hsl],
                                             va[bh][kt][:, h * 128:
                                                        (h + 1) * 128],
                                             ets[kt][:, hsl], start=(kt == 0),
                                             stop=(kt == MT - 1))
                    recD = act.tile([64, WD], BF, tag="recD", bufs=4,
                                    name="recD")
                    nc.vector.reciprocal(out=recD, in_=ps2[64:128, :])
                    hf = ((h // 2) % 2) * WD
                    nc.vector.tensor_tensor(
                        out=ct8r[h // 4][pi:pi + 64, hf:hf + WD],
                        in0=ps2[0:64, :], in1=recD, op=ALU.mult)
                ct8 = [t.rearrange("p (two n) -> p two n", two=2)
                       for t in ct8r]
                for mo in range(KT):
                    ps = pmm.tile([128, WD], F32, tag="mm", bufs=3,
                                  name="psh2")
                    for kb in range(2):
                        nc.tensor.matmul(
                            ps, wh_b[kb][:, :, mo * 128:(mo + 1) * 128],
                            ct8[kb], start=(kb == 0), stop=(kb == 1),
                            perf_mode=DR)
                    if i == 0:
                        nc.scalar.copy(out=h2sb[p][mo], in_=ps)
                    elif i < 3:
                        nc.vector.tensor_tensor(out=h2sb[p][mo],
                                                in0=h2sb[p][mo], in1=ps,
                                                op=ALU.add)
                    else:
                        nc.vector.scalar_tensor_tensor(
                            h2sb[p][mo], ps, bhat_t[mo], h2sb[p][mo],
                            op0=ALU.add, op1=ALU.add)

        # ---------------- Final: residual + LN2 + W2 ----------------
        w2_t = [wts.tile([128, H], BF, tag="w2", bufs=4, name="w2p")
                for _ in range(KT)]
        for k in range(KT):
            nc.sync.dma_start(out=w2_t[k], in_=W2p[k * 128:(k + 1) * 128, :])
        for p in range(PR):
            s2 = []
            for k in range(KT):
                s = act.tile([128, WD], BF, tag="s2", bufs=6, name="s2")
                nc.vector.tensor_tensor(out=s, in0=h2sb[p][k],
                                        in1=htTb[p][k], op=ALU.add)
                s2.append(s)
            n2 = [act.tile([128, WD], BF, tag="n2", bufs=6, name="n2")
                  for _ in range(KT)]
            layer_norm_T(s2, None, dests=n2)
            for bh in range(2):
                for mo in range(MT):
                    ps = pmm.tile([128, H], F32, tag="mm", bufs=3, name="pso")
                    for ki in range(KT):
                        off = bh * T + mo * 128
                        nc.tensor.matmul(ps, n2[ki][:, off:off + 128],
                                         w2_t[ki], start=(ki == 0),
                                         stop=(ki == KT - 1))
                    osb = act.tile([128, H], F32, tag="osb", bufs=2,
                                   name="osb")
                    if apply_c2:
                        nc.vector.tensor_tensor(out=osb, in0=ps, in1=c2n,
                                                op=ALU.add)
                    else:
                        nc.scalar.copy(out=osb, in_=ps)
                    nc.sync.dma_start(
                        out=out[2 * p + bh, mo * 128:(mo + 1) * 128, :],
                        in_=osb)
        pcx.release()
        psc.release()
        pmm.release()
        act.release()
        wts.release()
        per.release()
        cst.release()

    nc.compile()
    return nc


def _host_prep(inputs):
    f32 = np.float32
    E4 = ml_dtypes.float8_e4m3
    BF16 = ml_dtypes.bfloat16
    g = {}
    x = np.asarray(inputs["x"], f32)
    lengths = np.asarray(inputs["lengths"])
    speakers = np.asarray(inputs["speakers"])
    emo = np.asarray(inputs["emo_table"], f32)

    xTa = np.ascontiguousarray(x.transpose(0, 2, 1))  # [B, H, T]
    xTp = np.ascontiguousarray(
        xTa.reshape(B // 2, 2, H, T).transpose(0, 2, 1, 3).reshape(
            B // 2, H, WD))
    # fp8 DoubleRow rhs layout: [pr, kb, p, i*WD + j] = xTp[pr, 256kb+128i+p, j]
    xT8 = np.ascontiguousarray(
        xTp.astype(E4).reshape(B // 2, 2, 2, 128, WD).transpose(
            0, 1, 3, 2, 4).reshape(B // 2, 2, 128, 2 * WD))

    j = np.arange(T)
    row = (j[None, :] < lengths[:, None]).astype(f32)
    col = row
    sp = speakers.astype(f32)
    u1 = row * sp
    u2 = row * (1.0 - sp)
    ones = np.ones_like(row)
    sm = np.zeros((B, 3, 9, T), f32)
    sm[:, 0, 0] = NEG * (1.0 - col)                               # 0: FR glob
    sm[:, 0, 1], sm[:, 1, 1], sm[:, 2, 1] = ones, u1, u2          # 1: FL
    sm[:, 0, 2], sm[:, 1, 2], sm[:, 2, 2] = (NEG * ones, -NEG * sp,
                                             -NEG * (1.0 - sp))   # 2: FRintra
    sm[:, 0, 3], sm[:, 1, 3], sm[:, 2, 3] = ones, u1, u2          # 3: FL dup
    sm[:, 0, 4], sm[:, 1, 4], sm[:, 2, 4] = (NEG * ones,
                                             -NEG * (1.0 - sp) * col,
                                             -NEG * sp * col)     # 4: FRinter
    sm[:, 0, 5] = col                                             # 5
    sm[:, 0, 6] = row                                             # 6
    sm[:, 0, 7] = 1.0 - row                                       # 7
    sm[:, 0, 8] = ones[0]                                         # 8

    # combined-batch mask factor rows (one matmul covers both halves),
    # at 32-aligned partition offsets; see kernel-side layout comment.
    Lall = np.zeros((B // 2, 2, 128, T), f32)
    Rall = np.zeros((B // 2, 128, WD), f32)
    for pr in range(B // 2):
        for s_ in range(2):
            b = 2 * pr + s_
            hs = slice(s_ * T, (s_ + 1) * T)
            Lall[pr, 0, 0 + s_] = NEG * (1.0 - col[b])
            Rall[pr, 0 + s_, hs] = 1.0
            Lall[pr, 0, 32 + s_] = col[b]
            Rall[pr, 32 + s_, hs] = row[b]
            Lall[pr, 0, 64 + s_] = 1.0
            Rall[pr, 64 + s_, hs] = 1.0 - row[b]
            Lall[pr, 0, 96 + 3 * s_] = NEG
            Lall[pr, 0, 97 + 3 * s_] = -NEG * sp[b]
            Lall[pr, 0, 98 + 3 * s_] = -NEG * (1.0 - sp[b])
            Rall[pr, 96 + 3 * s_, hs] = 1.0
            Rall[pr, 97 + 3 * s_, hs] = u1[b]
            Rall[pr, 98 + 3 * s_, hs] = u2[b]
            Lall[pr, 1, 96 + 3 * s_] = NEG
            Lall[pr, 1, 97 + 3 * s_] = -NEG * (1.0 - sp[b]) * col[b]
            Lall[pr, 1, 98 + 3 * s_] = -NEG * sp[b] * col[b]

    band = (np.abs(j[:, None] - j[None, :]) <= 2)
    g["bandd"] = np.concatenate([band, band], axis=1).astype(BF16)
    kemo = (emo @ np.asarray(inputs["t_Wk"], f32)
            + np.asarray(inputs["t_bk"], f32))
    kTe32 = np.zeros((H, 32), f32)
    kTe32[:, 0:E] = kemo.T
    g["kTe32"] = kTe32.astype(BF16)
    vemo = (emo @ np.asarray(inputs["t_Wv"], f32)
            + np.asarray(inputs["t_bv"], f32))
    vaug = np.ones((E, VW), f32)
    vaug3 = vaug.reshape(E, NH, 128)
    vaug3[:, :, 0:64] = vemo.reshape(E, NH, 64)
    vea4 = np.zeros((128, VW), f32)
    for gg in range(4):
        vea4[32 * gg:32 * gg + E] = vaug
    g["vea4"] = vea4.astype(BF16)

    def pack_dr(W):
        A = np.asarray(W, f32).astype(E4)
        return A.reshape(2, 2, 128, H).transpose(0, 2, 1, 3).reshape(
            2, 128, 2 * H)

    isq = 1.0 / np.sqrt(DH)
    W1 = np.asarray(inputs["W1"], np.float64)
    bWo = np.asarray(inputs["b_Wo"], np.float64)
    What = [(bWo[i] @ W1[i * H:(i + 1) * H]).astype(f32) for i in range(4)]
    mats = [np.asarray(inputs["t_Wq"], f32) * isq,
            np.asarray(inputs["t_Wo"], f32)]
    for i in range(4):
        mats += [np.asarray(inputs["b_Wq"][i], f32) * isq,
                 np.asarray(inputs["b_Wk"][i], f32),
                 np.asarray(inputs["b_Wv"][i], f32),
                 What[i]]
    g["w8"] = np.stack([pack_dr(m) for m in mats])

    ln2g = np.asarray(inputs["ln2_g"], np.float64)
    g["W2p"] = (ln2g[:, None]
                * np.asarray(inputs["W2"], np.float64)).astype(BF16)
    g["onesb"] = np.ones(T, BF16)
    g["tbq"] = np.asarray(inputs["t_bq"], f32) * isq
    g["tbo"] = (np.asarray(inputs["t_bo"], np.float64)
                + np.asarray(inputs["t_bv"], np.float64)
                @ np.asarray(inputs["t_Wo"], np.float64)).astype(f32)
    g["g1"] = np.asarray(inputs["t_ln_g"], f32)
    g["b1v"] = np.asarray(inputs["t_ln_b"], f32)
    g["bbq"] = np.asarray(inputs["b_bq"], f32) * isq
    g["bbk"] = np.asarray(inputs["b_bk"], f32)
    bhat = np.asarray(inputs["b1"], np.float64).copy()
    for i in range(4):
        eff = (np.asarray(inputs["b_bo"][i], np.float64)
               + np.asarray(inputs["b_bv"][i], np.float64) @ bWo[i])
        bhat += eff @ W1[i * H:(i + 1) * H]
    g["bhat"] = bhat.astype(f32)
    g["c2row"] = (np.asarray(inputs["ln2_b"], np.float64)
                  @ np.asarray(inputs["W2"], np.float64)).astype(f32)

    apply_g1b1 = not (np.all(inputs["t_ln_g"] == 1.0)
                      and np.all(inputs["t_ln_b"] == 0.0))
    apply_c2 = bool(np.any(g["c2row"] != 0.0))

    Lb = Lall.astype(BF16)
    Rb = Rall.astype(BF16)
    in_maps = []
    for c in range(NCORES):
        m = dict(g)
        m["xT8"] = np.ascontiguousarray(xT8[c * PR:(c + 1) * PR])
        m["xTf"] = np.ascontiguousarray(xTp[c * PR:(c + 1) * PR]).astype(BF16)
        m["Lall"] = np.ascontiguousarray(Lb[c * PR:(c + 1) * PR])
        m["Rall"] = np.ascontiguousarray(Rb[c * PR:(c + 1) * PR])
        in_maps.append(m)
    return in_maps, apply_g1b1, apply_c2


def kernel(**inputs):
    in_maps, apply_g1b1, apply_c2 = _host_prep(inputs)
    key = (apply_g1b1, apply_c2)
    if key not in _CACHE:
        _CACHE[key] = _build(*key)
    nc = _CACHE[key]
    res = run_bass_kernel_spmd(nc, in_maps, core_ids=list(range(NCORES)),
                               trace=False)
    outs = [res.results[c]["out"] for c in range(NCORES)]
    return np.concatenate(outs, axis=0)
